# revision 18
# baseline (speedup 1.0000x reference)
"""Attention-pooling kernel for Trainium2 (8 NeuronCores, data-parallel).

Computes, for x:[B,T,D]=[64,4096,512] f32, W:[D,1], b:[T,1]:
    e = tanh(x @ W + b)          # [B,T,1]
    a = softmax(e, axis=1)       # over T
    out = sum(a * x, axis=1)     # [B,D]

Sharding: batch dim across 8 cores (8 samples/core); W,b replicated.

Design notes:
  * tanh bounds e to [-1,1], so softmax needs no max-subtraction pass ->
    single streaming pass over x (memory roofline = one x read ~ 64MiB/core).
  * natural [t,d] layout: partitions = 128 t-values, free = d (contiguous
    2KB per partition -> full DMA bandwidth), HWDGE (nc.sync) DMA.
  * e_t = b_t + sum_d x[t,d]*W[d]: fused custom-DVE TENSOR_TENSOR_REDUCE
    (accum = s0 + sum(in0*in1*s1)) per 512-chunk; a fraction of chunks is
    offloaded to GPSIMD(multiply) + ACT(activation accumulate) to keep DVE
    off the critical path (1-port DVE ops never contend with GPSIMD).
  * weighted sum out[d] = sum_t p_t x[t,d] is a PE matmul contracting the
    partition axis: psum[1,512] += p[128,1].T @ x[128,512], accumulated
    across all 32 t-chunks of a batch sample. float32r operands = full-rate
    fp32 on the PE (x is DMA'd as float32r bits; p is ACT-rounded).
  * s = sum_t p_t via free-dim reduce + ones-matmul partition sum;
    out = acc * (1/s) on ACT reading PSUM directly.
  * batch tails are emitted one batch late ("software pipelined") so the
    in-order ACT/DVE queues never stall waiting for the accumulation
    group of the current batch.
"""

import numpy as np

import concourse.bass as bass
import concourse.bacc as bacc
import concourse.mybir as mybir
from concourse.bass_utils import run_bass_kernel_spmd
from concourse.dve_ops import TENSOR_TENSOR_REDUCE
from concourse.tile import TileContext

B, T, D = 64, 4096, 512
N_CORES = 8
B_LOC = B // N_CORES  # 8 batch samples per core

T_TILE = 512                  # t-values per x tile
N_CHUNK = T_TILE // 128       # 4 column-chunks of 512 per tile
N_TILES = T // T_TILE         # 8 tiles per batch sample
NT = T // 128                 # 32 total t-chunks per batch sample

GPS_CHUNKS = (3,)             # chunks offloaded to GPSIMD+ACT per tile
X_BUFS = 16
CONTIG = True                 # contiguous-per-partition x tile DMA layout

FP32 = mybir.dt.float32
FP32R = mybir.dt.float32r


def build_bass(
    n_reps: int = 1,
    t_tile: int = T_TILE,
    gps_chunks: tuple = GPS_CHUNKS,
    x_bufs: int = X_BUFS,
    contig: bool = CONTIG,
) -> bass.Bass:
    """contig=True: x tile layout tile[p, j, d] = x[t0 + p*n_chunk + j, d]
    so each SBUF partition reads ONE contiguous (n_chunk*2KB) DRAM block per
    DMA (best DMA efficiency).  contig=False: tile[p, j, d] =
    x[t0 + j*128 + p, d] (n_chunk separate 2KB reads per partition).
    Both keep p/x chunk alignment, so the math is identical; only the
    (irrelevant) t-order within the softmax changes."""
    n_chunk = t_tile // 128
    n_tiles = T // t_tile
    nc = bacc.Bacc("TRN2", target_bir_lowering=False)

    x_h = nc.declare_dram_parameter("x", [B_LOC, T, D], FP32, isOutput=False)
    w_h = nc.declare_dram_parameter("W", [D, 1], FP32, isOutput=False)
    b_h = nc.declare_dram_parameter("b", [T, 1], FP32, isOutput=False)
    out_h = nc.declare_dram_parameter("out", [B_LOC, D], FP32, isOutput=True)

    with TileContext(nc) as tc:
        with (
            tc.tile_pool(name="consts", bufs=1) as consts,
            tc.tile_pool(name="xpool", bufs=x_bufs) as xpool,
            tc.tile_pool(name="tmp", bufs=3) as tmp,
            tc.tile_pool(name="ecols", bufs=4) as ecols,
            tc.tile_pool(name="pbatch", bufs=3) as pbatch,
            tc.tile_pool(name="small", bufs=3) as small,
            tc.tile_pool(name="acc_psum", bufs=3, space="PSUM") as acc_psum,
            tc.tile_pool(name="s_psum", bufs=3, space="PSUM") as s_psum,
        ):
            # --- constants ---
            # W broadcast to all 128 partitions: [128, 512]
            w_bcast = consts.tile([128, D], FP32)
            nc.gpsimd.dma_start(
                out=w_bcast[:], in_=bass.AP(w_h, 0, [[0, 128], [1, D]])
            )
            # b rearranged so column n holds b[128n : 128n+128]: [128, 32]
            b_sb = consts.tile([128, NT], FP32)
            if contig:
                b_ap = bass.AP(
                    b_h, 0, [[n_chunk, 128], [t_tile, n_tiles], [1, n_chunk]]
                )
            else:
                b_ap = bass.AP(b_h, 0, [[1, 128], [128, NT]])
            nc.gpsimd.dma_start(out=b_sb[:], in_=b_ap)
            ones = consts.tile([128, 1], FP32)
            nc.vector.memset(ones[:], 1.0)

            accs, ps = {}, {}

            def emit_tiles(bi_rep: int, bi: int):
                # p = exp(tanh(e)) for all 32 t-chunks of this sample
                p_all = pbatch.tile([128, NT], FP32R, name=f"p{bi_rep}", tag="p")
                acc = acc_psum.tile([1, D], FP32, name=f"acc{bi_rep}", tag="acc")
                accs[bi_rep], ps[bi_rep] = acc, p_all

                for ti in range(n_tiles):
                    # x tile [128, N_CHUNK, 512]: tile[p, j, d] =
                    # x[bi, ti*T_TILE + j*128 + p, d].  float32r so the PE
                    # matmuls run full-rate; f32-bitcast views feed DVE/GPS.
                    x_tile = xpool.tile(
                        [128, n_chunk, D], FP32R, name=f"x{bi_rep}_{ti}", tag="x"
                    )
                    off = (bi * T + ti * t_tile) * D
                    if contig:
                        x_ap = bass.AP(
                            x_h, off, [[n_chunk * D, 128], [D, n_chunk], [1, D]]
                        )
                    else:
                        x_ap = bass.AP(
                            x_h, off, [[D, 128], [128 * D, n_chunk], [1, D]]
                        )
                    nc.sync.dma_start(out=x_tile[:], in_=x_ap.bitcast(FP32R))

                    # e = b + x @ W, one column per 512-chunk
                    e_cols = ecols.tile(
                        [128, n_chunk], FP32, name=f"e{bi_rep}_{ti}", tag="e"
                    )
                    for j in range(n_chunk):
                        n = ti * n_chunk + j
                        if j in gps_chunks:
                            g_tmp = tmp.tile(
                                [128, D], FP32, tag="g_tmp", name=f"g{bi_rep}{ti}{j}"
                            )
                            nc.gpsimd.tensor_tensor(
                                out=g_tmp[:],
                                in0=x_tile[:, j, :].bitcast(FP32),
                                in1=w_bcast[:],
                                op=mybir.AluOpType.mult,
                            )
                            a_tmp = tmp.tile(
                                [128, 1], FP32, tag="a_tmp", name=f"a{bi_rep}{ti}{j}"
                            )
                            nc.scalar.activation(
                                out=a_tmp.broadcast_to([128, D]),
                                in_=g_tmp[:],
                                func=mybir.ActivationFunctionType.Copy,
                                accum_out=e_cols[:, j : j + 1],
                            )
                        else:
                            tt_out = tmp.tile(
                                [128, 1], FP32, tag="tt_out", name=f"t{bi_rep}{ti}{j}"
                            )
                            nc.vector._custom_dve(
                                TENSOR_TENSOR_REDUCE,
                                out=tt_out.broadcast_to([128, D]),
                                in0=x_tile[:, j, :].bitcast(FP32),
                                in1=w_bcast[:],
                                s0=b_sb[:, n : n + 1],
                                s1=1.0,
                                accum_out=e_cols[:, j : j + 1],
                            )
                    # ACT accumulate has no init term: add b for GPS columns
                    lo, hi = min(gps_chunks), max(gps_chunks) + 1
                    n0 = ti * n_chunk
                    nc.vector.tensor_add(
                        e_cols[:, lo:hi],
                        e_cols[:, lo:hi],
                        b_sb[:, n0 + lo : n0 + hi],
                    )

                    # p = exp(tanh(e)) (same ACT table set -> one load)
                    t_cols = ecols.tile(
                        [128, n_chunk], FP32, tag="tc", name=f"tc{bi_rep}{ti}"
                    )
                    nc.scalar.activation(
                        out=t_cols[:],
                        in_=e_cols[:],
                        func=mybir.ActivationFunctionType.Tanh,
                    )
                    nc.scalar.activation(
                        out=p_all[:, ti * n_chunk : (ti + 1) * n_chunk],
                        in_=t_cols[:],
                        func=mybir.ActivationFunctionType.Exp,
                    )

                    # acc[1, 512] += p_chunk.T @ x_chunk  (contract over t)
                    for j in range(n_chunk):
                        n = ti * n_chunk + j
                        nc.tensor.matmul(
                            acc[:, :],
                            lhsT=p_all[:, n : n + 1],
                            rhs=x_tile[:, j, :],
                            start=(n == 0),
                            stop=(n == NT - 1),
                        )

            def emit_tail(bi_rep: int, bi: int):
                p_all, acc = ps.pop(bi_rep), accs.pop(bi_rep)
                # s = sum_t p_t : free-dim reduce then partition-sum matmul
                s_col = small.tile([128, 1], FP32, tag="s_col", name=f"sc{bi_rep}")
                nc.vector.reduce_sum(
                    out=s_col[:], in_=p_all[:].bitcast(FP32), axis=mybir.AxisListType.X
                )
                s_ps = s_psum.tile([1, 1], FP32, tag="s_ps", name=f"sp{bi_rep}")
                nc.tensor.matmul(
                    s_ps[:, :], lhsT=s_col[:], rhs=ones[:], start=True, stop=True
                )
                inv_s = small.tile([1, 1], FP32, tag="inv_s", name=f"i{bi_rep}")
                nc.vector.reciprocal(out=inv_s[:], in_=s_ps[:, :])
                # out[bi] = acc * (1/s)
                out_sb = small.tile([1, D], FP32, tag="out_sb", name=f"o{bi_rep}")
                nc.scalar.mul(out_sb[:], acc[:, :], inv_s[:])
                nc.sync.dma_start(out=out_h[bi : bi + 1, :], in_=out_sb[:])

            # software-pipeline batches: emit batch k's tail after batch
            # k+1's tiles so tail deps are long-satisfied when the in-order
            # engine queues reach them.
            n_total = B_LOC * n_reps
            for k in range(n_total):
                emit_tiles(k, k % B_LOC)
                if k >= 1:
                    emit_tail(k - 1, (k - 1) % B_LOC)
            emit_tail(n_total - 1, (n_total - 1) % B_LOC)

    nc.compile()
    return nc


_CACHED_NC = None


def _get_nc():
    global _CACHED_NC
    if _CACHED_NC is None:
        _CACHED_NC = build_bass()
    return _CACHED_NC


def kernel(x: np.ndarray, W: np.ndarray, b: np.ndarray, **run_kwargs) -> np.ndarray:
    """Full-input entry point: shards over 8 cores, returns [B, D]."""
    x = np.ascontiguousarray(np.asarray(x, dtype=np.float32))
    W = np.ascontiguousarray(np.asarray(W, dtype=np.float32))
    b = np.ascontiguousarray(np.asarray(b, dtype=np.float32))
    assert x.shape == (B, T, D), x.shape

    nc = _get_nc()
    in_maps = [
        {"x": x[i * B_LOC : (i + 1) * B_LOC], "W": W, "b": b}
        for i in range(N_CORES)
    ]
    res = run_bass_kernel_spmd(nc, in_maps, core_ids=list(range(N_CORES)), **run_kwargs)
    out = np.concatenate([r["out"] for r in res.results], axis=0)
    return out


if __name__ == "__main__":
    rng = np.random.default_rng(0)
    x = rng.standard_normal((B, T, D), dtype=np.float32)
    W = rng.standard_normal((D, 1), dtype=np.float32)
    b = np.zeros((T, 1), dtype=np.float32)
    out = kernel(x=x, W=W, b=b)
    e = np.tanh(x.astype(np.float64) @ W + b)
    a = np.exp(e - e.max(axis=1, keepdims=True))
    a /= a.sum(axis=1, keepdims=True)
    ref = (a * x).sum(axis=1)
    rel = np.linalg.norm(out - ref) / np.linalg.norm(ref)
    print("Relative error vs numpy:", rel)
